# revision 1
# baseline (speedup 1.0000x reference)
"""Trainium2 Bass kernel for nn_BasicBlock (distance-transform conv BasicBlock).

Computes: relu(bn2(dt_conv2(relu(bn1(dt_conv1(x))))) + x)
where dt_conv is a 3x3 "distance transform conv":
    d[b,o,h,w] = sqrt(||p - c_o||^2),  p = 3x3 zero-padded patch (dim 576)

Strategy (8 NeuronCores, data-parallel over batch 32 -> 4 images/core):
- ||p||^2 - 2 p.c computed in ONE matmul accumulation group per pixel tile:
  SBUF partitions 0:64 hold x (weights = -2*centers), partitions 64:128 hold
  x^2 (weights = 1.0). 9 shifted matmuls (3x3 offsets) over a zero-padded
  plane accumulate in PSUM, K=128, M=64(out channels), N=392 pixels.
  This is contraction-optimal: 2*64ch*9offs = 1152 = 9 x K=128.
- Each image's padded plane is split into two half-image tiles (31 rows,
  2-row halo) so the first matmul only waits on a quarter of the input prep.
- Single-bank PSUM accumulation groups, 8 slots deep: maximal PSUM
  pipelining and the smallest possible evict/reduce quanta on the BN-stats
  critical path.
- d is stored 128-partitions wide (images 0,1 on partitions 0:64; images
  2,3 on 64:128 via partition-shifted eviction writes) so the final
  residual+relu runs at full 128-lane width. All per-channel scalars are
  duplicated across both halves, making partition-base questions moot.
- Eviction: d = sqrt(psum + ||c||^2) on ScalarE (per-partition bias), with
  accum_out giving per-channel sum(d) for BN stats for free. sum(d^2) =
  sum(psum) + n*||c||^2 via a VectorE reduce of the PSUM super-tile.
- Sync-BN: [64,2] per-channel stats AllGather across the 8 cores + local
  rank-sum (AG floor ~4.6us vs AR ~9.7us), x2 layers.
- Matmuls run in float32r (TF32-like, 1 col/cycle at N>=256, ~5e-6 rel err).

kernel(**inputs) takes FULL unsharded inputs, returns FULL output.
Self-contained: shapes/sharding hardcoded; no file reads.
"""
import numpy as np

from concourse import bacc, mybir, tile
from concourse.bass_utils import run_bass_kernel_spmd

f32 = mybir.dt.float32
f32r = mybir.dt.float32r
ADD = mybir.AluOpType.add
MULT = mybir.AluOpType.mult
SUB = mybir.AluOpType.subtract
AF = mybir.ActivationFunctionType

N_CORES = 8
B_LOCAL = 4            # images per core (32 / 8)
C = 64                 # channels (in == out)
HW = 56                # spatial
HP = HW + 2            # padded
RPG = 7                # rows per matmul group (N = 7*56 = 392)
GPS = 4                # groups per PSUM super-tile (4 banks)
N_SUPER = 2            # super-tiles (half-images) per image
HT = GPS * RPG + 3     # half-image tile rows: 28 out rows need 30 in + spare
N_ST = B_LOCAL * N_SUPER            # 8 super-tiles per layer
N_GLOBAL = 32 * HW * HW             # BN normalization count (global batch)
BN_EPS = 1e-5


def _pb(b):
    """Partition base and image index for d128/x128 layout."""
    return 64 * (b // 2), b % 2


def _build_layer(nc, psum, src, w, cst, ci, d, sumd, sumps):
    """One dt_conv layer: 32 single-bank groups x 9 offset matmuls (N=392).
    src[b][half] is a [128, HT, 58] padded half-image tile (x | x^2).
    d is [128, 2, HW, HW]; sumd/sumps [64, 32] (base partitions).
    Returns the eviction instructions (scheduling anchors)."""
    evicts = []
    NG = 4 * N_SUPER    # single-bank groups per image (7 rows each)
    for b in range(B_LOCAL):
        pb, i = _pb(b)
        for su in range(NG):
            half, within = su // 4, su % 4
            col = b * NG + su
            ps = psum.tile([C, 8, 64], f32, tag="ps")
            r0 = within * RPG
            for k in range(9):
                kh, kw = k // 3, k % 3
                nc.tensor.matmul(
                    ps[:, 0:RPG, 0:HW],
                    w[:, k, :],
                    src[b][half][:, r0 + kh:r0 + kh + RPG, kw:kw + HW],
                    start=(k == 0), stop=(k == 8),
                )
            rows = su * RPG
            # per-channel sum(psum) (-> sum(d^2) after +n*c2); emitted first
            # so it runs concurrently with the ACT eviction
            nc.vector.tensor_reduce(
                out=sumps[:, col:col + 1],
                in_=ps[:, 0:RPG, 0:HW],
                axis=mybir.AxisListType.XY, op=ADD)
            # d = sqrt(psum + ||c||^2); accum_out (always partitions 0:64)
            # gives the per-channel sum(d)
            ev = nc.scalar.activation(
                out=d[pb:pb + C, i, rows:rows + RPG, :],
                in_=ps[:, 0:RPG, 0:HW],
                func=AF.Sqrt, bias=cst[pb:pb + C, ci:ci + 1], scale=1.0,
                accum_out=sumd[:, col:col + 1])
            evicts.append(ev)
    return evicts


def _bn_affine(nc, pool, gstats, c2, gamma, beta, eps, name):
    """From [sum(d), sum(psum)] (dup both halves) -> scale s, shift t [128,1]."""
    P = 2 * C
    mu = pool.tile([P, 1], f32, tag=f"mu_{name}")
    ed2 = pool.tile([P, 1], f32, tag=f"ed2_{name}")
    mu2 = pool.tile([P, 1], f32, tag=f"mu2_{name}")
    var = pool.tile([P, 1], f32, tag=f"var_{name}")
    sd = pool.tile([P, 1], f32, tag=f"sd_{name}")
    inv = pool.tile([P, 1], f32, tag=f"inv_{name}")
    s = pool.tile([P, 1], f32, tag=f"s_{name}")
    st = pool.tile([P, 1], f32, tag=f"st_{name}")
    tt = pool.tile([P, 1], f32, tag=f"t_{name}")
    inv_n = 1.0 / float(N_GLOBAL)
    nc.vector.tensor_scalar_mul(out=mu[:, :], in0=gstats[:, 0:1], scalar1=inv_n)
    # E[d^2] = sum(psum)/N + c2  (DVE: keeps ACT out of the critical chain)
    nc.vector.scalar_tensor_tensor(
        out=ed2[:, :], in0=gstats[:, 1:2], scalar=inv_n, in1=c2,
        op0=MULT, op1=ADD)
    nc.vector.tensor_tensor(out=mu2[:, :], in0=mu[:, :], in1=mu[:, :], op=MULT)
    nc.vector.tensor_tensor(out=var[:, :], in0=ed2[:, :], in1=mu2[:, :], op=SUB)
    nc.scalar.activation(out=sd[:, :], in_=var[:, :], func=AF.Sqrt,
                         bias=eps[:, 0:1], scale=1.0)
    nc.vector.reciprocal(out=inv[:, :], in_=sd[:, :])
    nc.vector.tensor_tensor(out=s[:, :], in0=gamma, in1=inv[:, :], op=MULT)
    nc.vector.tensor_tensor(out=st[:, :], in0=mu[:, :], in1=s[:, :], op=MULT)
    nc.vector.tensor_tensor(out=tt[:, :], in0=beta, in1=st[:, :], op=SUB)
    return s, tt


def _stats_allreduce(nc, pool, dram, sumd, sumps, name, no_collective=False):
    """Reduce stat columns, AllGather [64,2] across 8 cores + local rank-sum,
    return [128,2] duplicated global sums."""
    red = pool.tile([C, 2], f32, tag=f"red_{name}")
    gstats = pool.tile([2 * C, 2], f32, tag=f"gstats_{name}")
    nc.vector.tensor_reduce(out=red[:, 0:1], in_=sumd[:, :],
                            axis=mybir.AxisListType.X, op=ADD)
    nc.vector.tensor_reduce(out=red[:, 1:2], in_=sumps[:, :],
                            axis=mybir.AxisListType.X, op=ADD)
    if no_collective:
        nc.vector.tensor_copy(out=gstats[0:C, :], in_=red[:, :])
        nc.vector.tensor_copy(out=gstats[C:2 * C, :], in_=gstats[0:C, :])
        return gstats
    cc_in = dram.tile([C, 2], f32, tag=f"ccin_{name}")
    # AllGather (floor ~4.6us vs AllReduce ~9.7us) + local rank-sum.
    # AG output is rank-major on the partition axis: [8*64, 2].
    cc_out = dram.tile([N_CORES * C, 2], f32, tag=f"ccout_{name}")
    gag = pool.tile([C, N_CORES, 2], f32, tag=f"gag_{name}")
    nc.sync.dma_start(out=cc_in[:, :], in_=red[:, :])
    nc.gpsimd.collective_compute(
        "AllGather", mybir.AluOpType.bypass,
        replica_groups=[list(range(N_CORES))],
        ins=[cc_in.opt()],
        outs=[cc_out.opt()],
    )
    # gag[c, r, s] <- cc_out[r*64 + c, s]
    nc.sync.dma_start(
        out=gag[:, :, :],
        in_=cc_out[:, :].rearrange("(r c) s -> c r s", r=N_CORES))
    nc.vector.tensor_reduce(out=gstats[0:C, 0:1], in_=gag[:, :, 0],
                            axis=mybir.AxisListType.X, op=ADD)
    nc.vector.tensor_reduce(out=gstats[0:C, 1:2], in_=gag[:, :, 1],
                            axis=mybir.AxisListType.X, op=ADD)
    # duplicate to the upper partition half (DVE partition-shift copy)
    nc.vector.tensor_copy(out=gstats[C:2 * C, :], in_=gstats[0:C, :])
    return gstats


def build(no_collective=False, reps=1):
    nc = bacc.Bacc("TRN2", target_bir_lowering=False, debug=False,
                   num_devices=1 if no_collective else N_CORES)
    x_ext = nc.declare_dram_parameter("x", [B_LOCAL, C, HW, HW], f32r, isOutput=False)
    w1_ext = nc.declare_dram_parameter("w1", [2 * C, 9, C], f32r, isOutput=False)
    w2_ext = nc.declare_dram_parameter("w2", [2 * C, 9, C], f32r, isOutput=False)
    # packed [c2a | c2b | g1 | b1 | g2 | b2], duplicated on both halves
    cst_ext = nc.declare_dram_parameter("cst", [2 * C, 6], f32, isOutput=False)
    out_ext = nc.declare_dram_parameter("out", [B_LOCAL, C, HW, HW], f32, isOutput=True)

    with tile.TileContext(nc) as tc:
        with (
            tc.tile_pool(name="big", bufs=1) as big,
            tc.tile_pool(name="small", bufs=1) as pool,
            tc.tile_pool(name="psum", bufs=8, space="PSUM") as psum,
            tc.tile_pool(name="dram", bufs=1, space="DRAM") as dram,
        ):
            w1 = pool.tile([2 * C, 9, C], f32r, tag="w1")
            w2 = pool.tile([2 * C, 9, C], f32r, tag="w2")
            cst = pool.tile([2 * C, 6], f32, tag="cst")
            g1, b1 = cst[:, 2:3], cst[:, 3:4]
            g2, b2 = cst[:, 4:5], cst[:, 5:6]
            eps = pool.tile([2 * C, 1], f32, tag="eps")
            nc.vector.memset(eps[:, :], BN_EPS)
            # constants via the gpsimd SWDGE ring (SP/ACT rings carry x)
            nc.gpsimd.dma_start(out=w1[:, :, :], in_=w1_ext[:, :, :])
            nc.gpsimd.dma_start(out=cst[:, :], in_=cst_ext[:, :])
            nc.gpsimd.dma_start(out=w2[:, :, :], in_=w2_ext[:, :, :])

            for r in range(reps):
                xp = [[big.tile([2 * C, HT, HP], f32r, tag=f"xp{b}_{su}",
                                name=f"xp{b}_{su}") for su in range(N_SUPER)]
                      for b in range(B_LOCAL)]
                yp = [[big.tile([2 * C, HT, HP], f32r, tag=f"yp{b}_{su}",
                                name=f"yp{b}_{su}") for su in range(N_SUPER)]
                      for b in range(B_LOCAL)]
                # d & residual x, 128-wide: partitions 0:64 = images 0,1;
                # partitions 64:128 = images 2,3 (index = b % 2)
                d = big.tile([2 * C, 2, HW, HW], f32, tag="d")
                xres = big.tile([2 * C, 2, HW, HW], f32, tag="xres")
                sumd1 = pool.tile([C, 4 * N_ST], f32, tag="sumd1")
                sumps1 = pool.tile([C, N_ST * GPS], f32, tag="sumps1")
                sumd2 = pool.tile([C, 4 * N_ST], f32, tag="sumd2")
                sumps2 = pool.tile([C, N_ST * GPS], f32, tag="sumps2")

                if r == 0:
                    # zero pad borders once (interior-only writes after this;
                    # full-plane squares keep zero borders zero).
                    # top half-tile (su=0): row 0 is the image top border.
                    # bottom half-tile (su=1): rows HT-2..HT are border/spare.
                    # x tiles on DVE (idle at start), y tiles on Pool.
                    for tp_pair, eng in ((xp, nc.vector), (yp, nc.gpsimd)):
                        for b in range(B_LOCAL):
                            t0, t1 = tp_pair[b]
                            eng.memset(t0[:, 0:1, :].bitcast(f32), 0.0)
                            eng.memset(t0[:, :, 0:1].bitcast(f32), 0.0)
                            eng.memset(t0[:, :, HP - 1:HP].bitcast(f32), 0.0)
                            eng.memset(t1[:, HT - 2:HT, :].bitcast(f32), 0.0)
                            eng.memset(t1[:, :, 0:1].bitcast(f32), 0.0)
                            eng.memset(t1[:, :, HP - 1:HP].bitcast(f32), 0.0)

                # ---- x into padded half-tiles; partition-shifted square ----
                # top: x rows 0..29 -> local rows 1..30
                # bottom: x rows 27..55 -> local rows 0..28
                dma_engines = [nc.sync, nc.scalar, nc.sync, nc.scalar]
                for b in range(B_LOCAL):
                    eng = dma_engines[b]
                    eng.dma_start(
                        out=xp[b][0][0:C, 1:31, 1:HW + 1],
                        in_=x_ext[b:b + 1, :, 0:30, :].transpose([1, 0, 2, 3]))
                    nc.scalar.activation(
                        out=xp[b][0][C:2 * C, :, :], in_=xp[b][0][0:C, :, :],
                        func=AF.Square)
                    eng.dma_start(
                        out=xp[b][1][0:C, 0:29, 1:HW + 1],
                        in_=x_ext[b:b + 1, :, 27:56, :].transpose([1, 0, 2, 3]))
                    nc.scalar.activation(
                        out=xp[b][1][C:2 * C, :, :], in_=xp[b][1][0:C, :, :],
                        func=AF.Square)
                # ---- layer 1 ----
                ev1 = _build_layer(nc, psum, xp, w1, cst, 0, d, sumd1, sumps1)

                # residual copy of x, 128-wide layout: needed only at the very
                # end, so order it after the L1 evictions (DMA engines idle
                # mid-layer; keeps it off the startup critical path)
                for b in range(B_LOCAL):
                    pb, i = _pb(b)
                    xr = nc.gpsimd.dma_start(
                        out=xres[pb:pb + C, i:i + 1, :, :],
                        in_=x_ext[b:b + 1, :, :, :].transpose([1, 0, 2, 3])
                            .bitcast(f32))
                    tile.add_dep_helper(xr.ins, ev1[8 * b].ins,
                                        reason="defer xres DMA past L1 start")
                gstats1 = _stats_allreduce(nc, pool, dram, sumd1, sumps1, "l1",
                                           no_collective)
                s1, t1 = _bn_affine(nc, pool, gstats1, cst[:, 0:1], g1, b1,
                                    eps, "l1")

                # ---- glue: y = relu(s1*d + t1); y^2 on upper partitions ----
                # top: y rows 0..29 -> local 1..30; bottom: y rows 27..55 -> 0..28
                for b in range(B_LOCAL):
                    pb, i = _pb(b)
                    nc.scalar.activation(
                        out=yp[b][0][0:C, 1:31, 1:HW + 1],
                        in_=d[pb:pb + C, i, 0:30, :],
                        func=AF.Relu, bias=t1[pb:pb + C, 0:1],
                        scale=s1[pb:pb + C, 0:1])
                    nc.vector.tensor_tensor(
                        out=yp[b][0][C:2 * C, :, :], in0=yp[b][0][0:C, :, :],
                        in1=yp[b][0][0:C, :, :], op=MULT)
                    nc.scalar.activation(
                        out=yp[b][1][0:C, 0:29, 1:HW + 1],
                        in_=d[pb:pb + C, i, 27:56, :],
                        func=AF.Relu, bias=t1[pb:pb + C, 0:1],
                        scale=s1[pb:pb + C, 0:1])
                    nc.vector.tensor_tensor(
                        out=yp[b][1][C:2 * C, :, :], in0=yp[b][1][0:C, :, :],
                        in1=yp[b][1][0:C, :, :], op=MULT)

                # ---- layer 2 ----
                _build_layer(nc, psum, yp, w2, cst, 1, d, sumd2, sumps2)
                gstats2 = _stats_allreduce(nc, pool, dram, sumd2, sumps2, "l2",
                                           no_collective)
                s2, t2 = _bn_affine(nc, pool, gstats2, cst[:, 1:2], g2, b2,
                                    eps, "l2")

                # ---- final: out = relu(s2*d + t2 + x), 128-wide; DMA out ----
                for i in range(2):
                    for su in range(N_SUPER):
                        rows = su * GPS * RPG
                        rs = slice(rows, rows + GPS * RPG)
                        nc.vector.scalar_tensor_tensor(
                            out=d[:, i, rs, :], in0=d[:, i, rs, :],
                            scalar=s2[:, 0:1], in1=xres[:, i, rs, :],
                            op0=MULT, op1=ADD)
                        nc.scalar.activation(
                            out=d[:, i, rs, :], in_=d[:, i, rs, :],
                            func=AF.Relu, bias=t2[:, 0:1], scale=1.0)
                        for half in range(2):
                            b = 2 * half + i
                            nc.sync.dma_start(
                                out=out_ext[b:b + 1, :, rs, :].transpose(
                                    [1, 0, 2, 3]),
                                in_=d[64 * half:64 * half + C, i:i + 1, rs, :])
    nc.compile()
    return nc


_NC_CACHE = None


def _get_nc():
    global _NC_CACHE
    if _NC_CACHE is None:
        _NC_CACHE = build()
    return _NC_CACHE


def _make_in_maps(x, centers1, gamma1, beta1, centers2, gamma2, beta2):
    def prep_w(centers):
        w = np.empty((2 * C, 9, C), np.float32)
        # centers: [o, d] with d = c*9 + k  ->  w[c, k, o] = -2*centers[o, 9c+k]
        w[:C] = -2.0 * np.ascontiguousarray(
            centers.reshape(C, C, 9).transpose(1, 2, 0))
        w[C:] = 1.0
        return w

    c1 = np.asarray(centers1, np.float32)
    c2 = np.asarray(centers2, np.float32)
    cst = np.stack([
        (c1 ** 2).sum(1), (c2 ** 2).sum(1),
        np.asarray(gamma1, np.float32), np.asarray(beta1, np.float32),
        np.asarray(gamma2, np.float32), np.asarray(beta2, np.float32),
    ], axis=1).astype(np.float32)
    cst = np.ascontiguousarray(np.tile(cst, (2, 1)))   # duplicate both halves
    common = {
        "w1": prep_w(c1),
        "w2": prep_w(c2),
        "cst": cst,
    }
    x = np.asarray(x, np.float32)
    in_maps = []
    for c in range(N_CORES):
        m = dict(common)
        m["x"] = np.ascontiguousarray(x[c * B_LOCAL:(c + 1) * B_LOCAL])
        in_maps.append(m)
    return in_maps


def _run(inputs, trace=False, **kw):
    nc = _get_nc()
    in_maps = _make_in_maps(**inputs)
    res = run_bass_kernel_spmd(nc, in_maps, core_ids=list(range(N_CORES)),
                               trace=trace, **kw)
    out = np.concatenate([res.results[c]["out"] for c in range(N_CORES)], axis=0)
    return out.astype(np.float32), res


def kernel(**inputs):
    out, _ = _run(inputs)
    return out

